# revision 12
# baseline (speedup 1.0000x reference)
"""Expert-choice MoE router on 8 Trainium2 NeuronCores.

Sharding: data-parallel over the batch dim (B=8 rows -> 8 cores). Each core
computes its row's full MLP router (Linear(4096,1024) -> exact GELU ->
Linear(1024,1) -> sigmoid) plus the per-row variable-k top-k selection.

The big matmul runs as a 3-pass fp16 hi/lo split (hi*hi + hi*lo + lo*hi) on
the PE array: fp16 products are exact in fp32 accumulation, so the dropped
lo*lo term (~2^-22 relative) keeps the logit error ~1e-7 — far below the
~5e-5 minimum top-k boundary gap — while running at the full 1-cycle/row PE
rate instead of fp32's 4 cycles/row.

Top-k uses a fixed 26-step threshold bisection on masked logits (selection by
logit order == selection by sigmoid-score order): count(logit > t) computed on
DVE with a 32x32-transpose partition reduction, no host round trips.
"""
import numpy as np

import bass_rust
import concourse.bass as bass
import concourse.mybir as mybir
import concourse.tile as tile
from concourse.bass_utils import run_bass_kernel_spmd

B, S, D, H = 8, 4096, 4096, 1024
KC = D // 128          # 32 contraction chunks
HC = H // 128          # 8 hidden chunks
TT = 512               # token tile (free dim of mm1)
NT = S // TT           # 8 token tiles
N_ITER = 26            # bisection steps: 16 * 2^-26 = 2.4e-7 interval
LOGIT_BOUND = 8.0

F32 = mybir.dt.float32
F16 = mybir.dt.float16
U8 = mybir.dt.uint8
I32 = mybir.dt.int32
AF = mybir.ActivationFunctionType
ALU = mybir.AluOpType


def _install_drain_split_patch():
    """The installed walrus build accepts fewer sync waits per instruction
    than bass/Tile emits; split multi-wait instructions into single-wait NOPs."""
    if getattr(tile.TileContext, "_drain_split_patched", False):
        return

    def split_multi_waits(nc, max_waits=1):
        ctr = 0
        for fn in nc.m.functions:
            for blk in fn.blocks:
                new = []
                changed = False
                for inst in blk.instructions:
                    si = inst.sync_info
                    waits = list(si.on_wait) if si is not None and si.on_wait else []
                    if len(waits) > max_waits:
                        for w in waits[:-max_waits]:
                            ctr += 1
                            new.append(mybir.InstNoOp(
                                name=f"WS-{ctr}",
                                engine=inst.engine,
                                sync_info=mybir.SyncInfo(on_wait=[w], on_update=[]),
                                bass_nofuse=True,
                            ))
                        si.on_wait = waits[-max_waits:]
                        changed = True
                    new.append(inst)
                if changed:
                    blk.instructions = new

    orig = tile.TileContext._drain_and_barrier

    def patched(self, tick_clock, wait_clock):
        orig(self, tick_clock, wait_clock)
        split_multi_waits(self.nc)

    tile.TileContext._drain_and_barrier = patched
    tile.TileContext._drain_split_patched = True


def build_program(passes=3, do_mm2=True, do_tail=True, n_iter=N_ITER):
    _install_drain_split_patch()
    nc = bass.Bass()

    hs_t = nc.dram_tensor("hs_t", [D, S], F32, kind="ExternalInput")
    w1hi = nc.dram_tensor("w1hi", [D, H], F16, kind="ExternalInput")
    w1lo = nc.dram_tensor("w1lo", [D, H], F16, kind="ExternalInput")
    b1pk = nc.dram_tensor("b1pk", [128, HC], F32, kind="ExternalInput")
    w2pk = nc.dram_tensor("w2pk", [128, HC], F32, kind="ExternalInput")
    b2rep = nc.dram_tensor("b2rep", [32, 1], F32, kind="ExternalInput")
    am_t = nc.dram_tensor("am_t", [32, 128], U8, kind="ExternalInput")

    o_rw = nc.dram_tensor("o_rw", [S], F32, kind="ExternalOutput")
    o_sel = nc.dram_tensor("o_sel", [S], U8, kind="ExternalOutput")
    logit_dram = nc.dram_tensor("logit_scratch", [S], F32, kind="Internal")

    with tile.TileContext(nc) as tc:
        with (
            tc.tile_pool(name="wres", bufs=1) as wres,
            tc.tile_pool(name="xin", bufs=3) as xin,
            tc.tile_pool(name="hact", bufs=2) as hpool,
            tc.tile_pool(name="ps", bufs=8, space="PSUM") as ps,
            tc.tile_pool(name="tail", bufs=2) as tp,
        ):
            # resident weights
            w1hi_sb = wres.tile([128, KC, H], F16)
            nc.sync.dma_start(w1hi_sb[:], w1hi.rearrange("(k p) h -> p k h", p=128))
            w1lo_sb = wres.tile([128, KC, H], F16)
            nc.sync.dma_start(w1lo_sb[:], w1lo.rearrange("(k p) h -> p k h", p=128))
            b1_sb = wres.tile([128, HC], F32)
            nc.sync.dma_start(b1_sb[:], b1pk[:])
            w2_sb = wres.tile([128, HC], F32)
            nc.sync.dma_start(w2_sb[:], w2pk[:])
            b2_sb = wres.tile([32, 1], F32)
            nc.sync.dma_start(b2_sb[:], b2rep[:])
            am_sb = wres.tile([32, 128], U8)
            nc.sync.dma_start(am_sb[:], am_t[:])

            logits128 = wres.tile([128, NT * 4], F32)

            hs_v = hs_t.rearrange("(k p) t -> k p t", p=128)

            for T in range(NT):
                psum = [ps.tile([128, TT], F32, tag="ps", name=f"psum{T}_{h}")
                        for h in range(HC)]
                for k in range(KC):
                    xf = xin.tile([128, TT], F32, tag="xf")
                    nc.sync.dma_start(xf[:], hs_v[k, :, T * TT:(T + 1) * TT])
                    xhi = xin.tile([128, TT], F16, tag="xhi")
                    nc.scalar.copy(xhi[:], xf[:])
                    xlo = xin.tile([128, TT], F16, tag="xlo")
                    nc.vector.tensor_sub(xlo[:], xf[:], xhi[:])
                    for h in range(HC):
                        whi = w1hi_sb[:, k, h * 128:(h + 1) * 128]
                        wlo = w1lo_sb[:, k, h * 128:(h + 1) * 128]
                        nc.tensor.matmul(psum[h][:], whi, xhi[:],
                                         start=(k == 0), stop=(passes == 1 and k == KC - 1))
                        if passes >= 2:
                            nc.tensor.matmul(psum[h][:], whi, xlo[:],
                                             start=False, stop=(passes == 2 and k == KC - 1))
                        if passes >= 3:
                            nc.tensor.matmul(psum[h][:], wlo, xhi[:],
                                             start=False, stop=(k == KC - 1))
                hacts = []
                for h in range(HC):
                    ha = hpool.tile([128, TT], F32, tag=f"ha{h}")
                    nc.scalar.activation(ha[:], psum[h][:], AF.Gelu,
                                         bias=b1_sb[:, h:h + 1])
                    hacts.append(ha)
                if do_mm2:
                    ps2 = ps.tile([128, 4], F32, tag="ps")
                    for c in range(4):
                        for h in range(HC):
                            nc.tensor.matmul(
                                ps2[:, c:c + 1],
                                hacts[h][:, c * 128:(c + 1) * 128],
                                w2_sb[:, h:h + 1],
                                start=(h == 0), stop=(h == HC - 1))
                    nc.vector.tensor_copy(logits128[:, T * 4:(T + 1) * 4], ps2[:])
                else:
                    nc.vector.tensor_copy(logits128[:, T * 4:(T + 1) * 4],
                                          hacts[0][:, 0:4])

            if not do_tail:
                nc.sync.dma_start(o_rw.rearrange("(j p) -> p j", p=128), logits128[:])
                o_sel_v = o_sel.rearrange("(f q) -> q f", q=32)
                nc.sync.dma_start(o_sel_v, am_sb[:])
                return nc

            # reorder [128, 32] (token = col*128 + p) -> [32, 128] (token = col*32 + p)
            nc.sync.dma_start(logit_dram.rearrange("(j p) -> p j", p=128), logits128[:])
            lg = tp.tile([32, 128], F32, tag="lg")
            nc.sync.dma_start(lg[:], logit_dram.rearrange("(f q) -> q f", q=32))

            # masked logits
            mkf = tp.tile([32, 128], F32, tag="mkf")
            nc.vector.tensor_copy(mkf[:], am_sb[:])
            negbig = tp.tile([32, 128], F32, tag="negbig")
            nc.vector.memset(negbig[:], -1.0e30)
            ml = tp.tile([32, 128], F32, tag="ml")
            nc.vector.select(ml[:], am_sb[:], lg[:], negbig[:])

            # num_active (replicated across the 32 partitions)
            def preduce(src128):  # [32, 1] partials -> [32, 1] replicated total
                bc = tp.tile([32, 32], F32, tag="bc")
                nc.vector.tensor_copy(bc[:], src128.broadcast_to([32, 32]))
                tr = tp.tile([32, 32], F32, tag="tr")
                nc.vector.transpose(tr[:], bc[:])
                tot = tp.tile([32, 1], F32, tag="tot")
                nc.vector.reduce_sum(tot[:], tr[:], axis=mybir.AxisListType.X)
                return tot

            pc_na = tp.tile([32, 1], F32, tag="pc")
            nc.vector.reduce_sum(pc_na[:], mkf[:], axis=mybir.AxisListType.X)
            na = preduce(pc_na[:])

            # k = min(max(rne(0.5*na - 0.25), 1), na)   (== clamped floor)
            k0f = tp.tile([32, 1], F32, tag="k0f")
            nc.vector.tensor_scalar(k0f[:], na[:], 0.5, -0.25,
                                    op0=ALU.mult, op1=ALU.add)
            k0i = tp.tile([32, 1], I32, tag="k0i")
            nc.vector.tensor_copy(k0i[:], k0f[:])
            kf = tp.tile([32, 1], F32, tag="kf")
            nc.vector.tensor_copy(kf[:], k0i[:])
            nc.vector.tensor_scalar_max(kf[:], kf[:], 1.0)
            krep = tp.tile([32, 1], F32, tag="krep")
            nc.vector.tensor_tensor(krep[:], kf[:], na[:], op=ALU.min)

            # threshold bisection on masked logits
            lo = tp.tile([32, 1], F32, tag="lo")
            nc.vector.memset(lo[:], -LOGIT_BOUND)
            hi = tp.tile([32, 1], F32, tag="hi")
            nc.vector.memset(hi[:], LOGIT_BOUND)
            ge_scr = tp.tile([32, 128], F32, tag="ge_scr")
            for it in range(n_iter):
                mid = tp.tile([32, 1], F32, tag="mid")
                nc.vector.tensor_tensor(mid[:], lo[:], hi[:], op=ALU.add)
                nc.vector.tensor_scalar_mul(mid[:], mid[:], 0.5)
                pc = tp.tile([32, 1], F32, tag="pc")
                nc.vector.tensor_scalar(ge_scr[:], ml[:], mid[:], None,
                                        op0=ALU.is_gt)
                nc.vector.reduce_sum(pc[:], ge_scr[:], axis=mybir.AxisListType.X)
                cnt = preduce(pc[:])
                gek = tp.tile([32, 1], U8, tag="gek")
                nc.vector.tensor_tensor(gek[:], cnt[:], krep[:], op=ALU.is_ge)
                nlo = tp.tile([32, 1], F32, tag="lo")
                nc.vector.select(nlo[:], gek[:], mid[:], lo[:])
                nhi = tp.tile([32, 1], F32, tag="hi")
                nc.vector.select(nhi[:], gek[:], hi[:], mid[:])
                lo, hi = nlo, nhi

            sel = tp.tile([32, 128], F32, tag="sel")
            nc.vector.tensor_scalar(sel[:], ml[:], lo[:], None, op0=ALU.is_gt)

            scores = tp.tile([32, 128], F32, tag="scores")
            nc.scalar.activation(scores[:], lg[:], AF.Sigmoid, bias=b2_sb[:, 0:1])
            rw = tp.tile([32, 128], F32, tag="rw")
            nc.vector.tensor_mul(rw[:], scores[:], sel[:])
            sel8 = tp.tile([32, 128], U8, tag="sel8")
            nc.vector.tensor_copy(sel8[:], sel[:])

            nc.sync.dma_start(o_rw.rearrange("(f q) -> q f", q=32), rw[:])
            nc.sync.dma_start(o_sel.rearrange("(f q) -> q f", q=32), sel8[:])

    return nc


_NC_CACHE = {}


def _get_program():
    if "nc" not in _NC_CACHE:
        _NC_CACHE["nc"] = build_program()
    return _NC_CACHE["nc"]


def kernel(hidden_states, active_mask, W1, b1, W2, b2):
    hidden_states = np.asarray(hidden_states, dtype=np.float32)
    active_mask = np.asarray(active_mask)
    W1 = np.asarray(W1, dtype=np.float32)
    b1 = np.asarray(b1, dtype=np.float32)
    W2 = np.asarray(W2, dtype=np.float32)
    b2 = np.asarray(b2, dtype=np.float32)

    w1hi = W1.astype(np.float16)
    w1lo = (W1 - w1hi.astype(np.float32)).astype(np.float16)
    b1pk = np.ascontiguousarray(b1.reshape(HC, 128).T)
    w2pk = np.ascontiguousarray(W2[:, 0].reshape(HC, 128).T)
    b2rep = np.full((32, 1), b2[0], dtype=np.float32)

    in_maps = []
    for b in range(B):
        in_maps.append({
            "hs_t": np.ascontiguousarray(hidden_states[b].T),
            "w1hi": w1hi,
            "w1lo": w1lo,
            "b1pk": b1pk,
            "w2pk": w2pk,
            "b2rep": b2rep,
            "am_t": np.ascontiguousarray(
                active_mask[b].astype(np.uint8).reshape(128, 32).T),
        })

    nc = _get_program()
    res = run_bass_kernel_spmd(nc, in_maps, core_ids=list(range(B)))
    _NC_CACHE["last_results"] = res

    router_weights = np.stack([res.results[b]["o_rw"] for b in range(B)])
    selected_mask = np.stack([res.results[b]["o_sel"] for b in range(B)]).astype(bool)
    return router_weights, selected_mask
